# revision 1
# baseline (speedup 1.0000x reference)
"""Trainium2 Bass kernel for nn_DotAttentionLayer (edge-parallel sparse GNN attention).

Self-contained: takes FULL inputs (query, memory, edge_rows, edge_cols, Wq, bq,
Wk, bk, Wv, bv, Wo, bo), returns the FULL [50000, 128] output.

Strategy (8 cores, SPMD):
  - Edges sharded by row-owner core (row // 6250); node tables replicated.
  - Host colors each core's edges into 1536-edge tiles such that within a tile
    all rows are unique AND all cols are unique (scatter-add duplicates within
    one DMA instruction race on HW; across instructions Tile's WAW serializes).
  - Gathers use the MoE dma_gather ucode (int16 idx): col indices use a
    paired-row view [25088, 256] with idx=col>>1 (+ on-device parity select)
    to stay under the int16 limit; row indices are core-local (< 6272).
  - Pass 1: gather q,k rows per edge, dot per head, exp -> den scatter-add
    (striped, col-half split at 25088).  AllReduce den (800KB).  vn = v/den
    folds softmax denominators into the value table.  Pass 2: gather vn,
    msg = ex * vn, scatter-add into striped res tables.  out = res @ Wo.
"""

import sys

sys.path.insert(0, "/opt/trn_rl_repo")
import numpy as np

import concourse.bacc as bacc
import concourse.mybir as mybir
import concourse.tile as tile
from concourse import library_config
from concourse.bass import AP  # noqa
from concourse.masks import make_identity

# ---------------- problem constants ----------------
N = 50000
M = 50000
E = 1000000
F = 128
H = 4
D = 32
SCALE = float(1.0 / np.sqrt(D))
LRELU_ALPHA = 0.2

CORES = 8
RPC = N // CORES            # 6250 rows per core
QROWS = 6272                # 49*128, row-gather table size (trash row = RPC)
Q_TRASH = RPC
NODEP = 50176               # 392*128 padded node count
SPLIT = 25088               # col half split (196*128)
PAIRS = NODEP // 2          # 25088 pair rows
PAIR_DUMMY = N // 2         # pair of padded zero rows (50000,50001)
DENROWS = 25216             # 197*128; trash row below
DEN_TRASH_LO = SPLIT        # local trash row for lo half
DEN_TRASH_HI = 25000        # local trash row for hi half (real hi locals < 24912)
T = 2048                    # edges per tile
GC = 1024                   # max indices per dma_gather call (HW ring limit)
S_DEN = 2                   # den stripes per col half
S_RES = 4                   # res stripes
FP = mybir.dt.float32
I16 = mybir.dt.int16


# ---------------- host-side edge coloring ----------------
def _first_occ_mask(x):
    """Boolean mask of first occurrences in x."""
    _, idx = np.unique(x, return_index=True)
    m = np.zeros(len(x), bool)
    m[idx] = True
    return m


def _color_half(rows, cols, cap=T):
    """Greedy rounds: each tile gets edges with unique rows and unique cols.
    Returns list of np.ndarray of edge indices (into rows/cols)."""
    remaining = np.arange(len(rows))
    tiles = []
    while len(remaining):
        r = rows[remaining]
        c = cols[remaining]
        sel = _first_occ_mask(r) & _first_occ_mask(c)
        pick = remaining[sel][:cap]
        tiles.append(pick)
        keep = np.ones(len(remaining), bool)
        # indices of pick within remaining: sel positions limited to cap
        pos = np.nonzero(sel)[0][: len(pick)]
        keep[pos] = False
        remaining = remaining[keep]
    # tail packing: merge small tiles into earlier non-full ones
    tiles.sort(key=len, reverse=True)
    packed = []
    rowsets, colsets = [], []
    for tl in tiles:
        tr, tc = rows[tl], cols[tl]
        placed = False
        for i in range(len(packed)):
            if len(packed[i]) + len(tl) <= cap and \
               not np.any(np.isin(tr, rowsets[i], assume_unique=False)) and \
               not np.any(np.isin(tc, colsets[i], assume_unique=False)):
                packed[i] = np.concatenate([packed[i], tl])
                rowsets[i] = np.concatenate([rowsets[i], tr])
                colsets[i] = np.concatenate([colsets[i], tc])
                placed = True
                break
        if not placed:
            packed.append(tl)
            rowsets.append(tr)
            colsets.append(tc)
    return packed


def _wrap16(idx_2d):
    """[nt, T] -> [nt, 128, T//16] int16 wrap-16 layout replicated to 128."""
    nt = idx_2d.shape[0]
    w = idx_2d.reshape(nt, T // 16, 16).transpose(0, 2, 1)  # [nt,16,T//16]
    return np.tile(w, (1, 8, 1)).astype(np.int16)


def _edge_major(x_2d):
    """[nt, T] -> [nt, 128, T//128] float32 edge-major layout."""
    nt = x_2d.shape[0]
    return x_2d.reshape(nt, T // 128, 128).transpose(0, 2, 1).astype(np.float32)


def host_prep(edge_rows, edge_cols):
    """Shard + color edges. Returns per-core index arrays + (NT_LO, NT_HI)."""
    edge_rows = np.asarray(edge_rows)
    edge_cols = np.asarray(edge_cols)
    owner = edge_rows // RPC
    per_core = []
    for c in range(CORES):
        m = owner == c
        r = (edge_rows[m] - c * RPC).astype(np.int64)
        col = edge_cols[m].astype(np.int64)
        lo = col < SPLIT
        tiles_lo = _color_half(r[lo], col[lo])
        tiles_hi = _color_half(r[~lo], col[~lo])
        per_core.append((r, col, lo, tiles_lo, tiles_hi))
    nt_lo = max(len(p[3]) for p in per_core)
    nt_hi = max(len(p[4]) for p in per_core)

    inputs = []
    for c in range(CORES):
        r, col, lo, tiles_lo, tiles_hi = per_core[c]
        idx_lo = np.nonzero(lo)[0]
        idx_hi = np.nonzero(~lo)[0]
        nt = nt_lo + nt_hi
        rl = np.full((nt, T), Q_TRASH, np.int64)
        cg = np.full((nt, T), 2 * PAIR_DUMMY, np.int64)  # dummy global col 50000
        dl = np.zeros((nt, T), np.int64)
        for t in range(nt_lo):
            dl[t] = DEN_TRASH_LO
        for t in range(nt_hi):
            dl[nt_lo + t] = DEN_TRASH_HI
        for t, tl in enumerate(tiles_lo):
            e = idx_lo[tl]
            rl[t, : len(e)] = r[e]
            cg[t, : len(e)] = col[e]
            dl[t, : len(e)] = col[e]
        for t, tl in enumerate(tiles_hi):
            e = idx_hi[tl]
            rl[nt_lo + t, : len(e)] = r[e]
            cg[nt_lo + t, : len(e)] = col[e]
            dl[nt_lo + t, : len(e)] = col[e] - SPLIT
        inputs.append(
            dict(
                qg_idx=_wrap16(rl),
                kp_idx=_wrap16(cg >> 1),
                den_idx=_wrap16(dl),
                par=_edge_major((cg & 1).astype(np.float32)),
            )
        )
    return inputs, nt_lo, nt_hi


# ---------------- device program ----------------
def build_nc(nt_lo, nt_hi, with_bias, stage="full", rep=1):
    NT = nt_lo + nt_hi
    nc = bacc.Bacc("TRN2", debug=False)
    dbg = None
    if stage == "dense" or stage == "vn":
        dbg = nc.dram_tensor("dbg", [NODEP, F], FP, kind="ExternalOutput")
    elif stage in ("pass1", "ar"):
        dbg = nc.dram_tensor("dbg", [2 * SPLIT, 4], FP, kind="ExternalOutput")

    # I/O
    queryT = nc.dram_tensor("queryT", [F, QROWS], FP, kind="ExternalInput")
    memoryT = nc.dram_tensor("memoryT", [F, NODEP], FP, kind="ExternalInput")
    Wq = nc.dram_tensor("Wq", [F, F], FP, kind="ExternalInput")
    Wk = nc.dram_tensor("Wk", [F, F], FP, kind="ExternalInput")
    Wv = nc.dram_tensor("Wv", [F, F], FP, kind="ExternalInput")
    Wo = nc.dram_tensor("Wo", [F, F], FP, kind="ExternalInput")
    biases = nc.dram_tensor("biases", [4, F], FP, kind="ExternalInput")
    qg_idx = nc.dram_tensor("qg_idx", [NT, 128, T // 16], I16, kind="ExternalInput")
    kp_idx = nc.dram_tensor("kp_idx", [NT, 128, T // 16], I16, kind="ExternalInput")
    den_idx = nc.dram_tensor("den_idx", [NT, 128, T // 16], I16, kind="ExternalInput")
    par_in = nc.dram_tensor("par", [NT, 128, T // 128], FP, kind="ExternalInput")
    out_c = nc.dram_tensor("out", [QROWS, F], FP, kind="ExternalOutput")

    # internal DRAM
    q_tab = nc.dram_tensor("q_tab", [QROWS, F], FP)
    k_tab = nc.dram_tensor("k_tab", [NODEP, F], FP)
    v_tab = nc.dram_tensor("v_tab", [NODEP, F], FP)
    vn_tab = nc.dram_tensor("vn_tab", [NODEP, F], FP)
    den_st = [
        [nc.dram_tensor(f"den_{h}_{s}", [DENROWS, 64], FP) for s in range(S_DEN)]
        for h in range(2)
    ]
    den4 = nc.dram_tensor("den4", [2 * SPLIT, 4], FP)
    den4_ar = nc.dram_tensor("den4_ar", [2 * SPLIT, 4], FP)
    res_st = [nc.dram_tensor(f"res_{s}", [QROWS, F], FP) for s in range(S_RES)]

    with tile.TileContext(nc) as tc:
        with tc.tile_pool(name="static", bufs=1) as st, \
             tc.tile_pool(name="work", bufs=2) as wk, \
             tc.tile_pool(name="exp", bufs=1) as exp_pool, \
             tc.tile_pool(name="psum", bufs=3, space="PSUM") as pp, \
             tc.tile_pool(name="psum2", bufs=2, space="PSUM") as pp2:
            nc.gpsimd.load_library(library_config.mlp)

            # ---- static tiles ----
            zt = st.tile([128, 2048], FP, tag="zero")
            nc.vector.memset(zt[:], 0.0)
            ident = st.tile([128, 128], FP, tag="ident")
            make_identity(nc, ident[:])
            wq_t = st.tile([F, F], FP, tag="wq")
            wk_t = st.tile([F, F], FP, tag="wk")
            wv_t = st.tile([F, F], FP, tag="wv")
            wo_t = st.tile([F, F], FP, tag="wo")
            nc.sync.dma_start(out=wq_t[:], in_=Wq[:])
            nc.sync.dma_start(out=wk_t[:], in_=Wk[:])
            nc.sync.dma_start(out=wv_t[:], in_=Wv[:])
            nc.sync.dma_start(out=wo_t[:], in_=Wo[:])
            if with_bias:
                ones1 = st.tile([1, 128], FP, tag="ones1")
                nc.vector.memset(ones1[:], 1.0)
                bias_t = st.tile([4, F], FP, tag="bias")
                nc.sync.dma_start(out=bias_t[:], in_=biases[:])

            # ---- zero the accumulator stripes ----
            def zero_dram(tensor, rows, width):
                flat = rows * width  # multiple of 128*?
                per = flat // 128
                ap = tensor[:].rearrange("(a b) c -> a (b c)", a=128)
                off = 0
                while off < per:
                    n = min(2048, per - off)
                    nc.sync.dma_start(out=ap[:, off:off + n], in_=zt[:, 0:n])
                    off += n

            for h in range(2):
                for s in range(S_DEN):
                    zero_dram(den_st[h][s], DENROWS, 64)
            for s in range(S_RES):
                zero_dram(res_st[s], QROWS, F)

            # ---- dense phase: q/k/v tables ----
            def dense(srcT, ncols, w_ts, b_rows, dsts):
                for j0 in range(0, ncols, 512):
                    nn_ = min(512, ncols - j0)
                    xt = wk.tile([128, 512], FP, tag="xT")
                    nc.sync.dma_start(out=xt[:, 0:nn_], in_=srcT[:, j0:j0 + nn_])
                    for wi, (w_t, b_row, dst) in enumerate(zip(w_ts, b_rows, dsts)):
                        sb = wk.tile([128, 4, 128], FP, tag=f"dsb{wi}")
                        for cch in range(nn_ // 128):
                            ps = pp.tile([128, 128], FP, tag="dps")
                            nc.tensor.matmul(
                                out=ps[:],
                                lhsT=xt[:, cch * 128:(cch + 1) * 128],
                                rhs=w_t[:],
                                start=True, stop=not with_bias,
                            )
                            if with_bias:
                                nc.tensor.matmul(
                                    out=ps[:], lhsT=ones1[:], rhs=b_row,
                                    start=False, stop=True,
                                )
                            # lrelu(x) = max(0.2*x, x)
                            cp = wk.tile([128, 128], FP, tag="lrcp")
                            nc.scalar.copy(out=cp[:], in_=ps[:])
                            nc.vector.scalar_tensor_tensor(
                                out=sb[:, cch], in0=ps[:], scalar=LRELU_ALPHA,
                                in1=cp[:], op0=mybir.AluOpType.mult,
                                op1=mybir.AluOpType.max)
                        nc.sync.dma_start(
                            out=dst[j0:j0 + nn_, :].rearrange(
                                "(a b) c -> b a c", b=128),
                            in_=sb[:, 0:nn_ // 128],
                        )

            dense(queryT, QROWS, [wq_t], [biases[0:1] if with_bias else None],
                  [q_tab])
            dense(memoryT, NODEP, [wk_t, wv_t],
                  [biases[1:2] if with_bias else None,
                   biases[2:3] if with_bias else None],
                  [k_tab, v_tab])

            done = stage in ("dense", "tdense", "noop")
            if stage == "dense":
                nc.sync.dma_start(out=dbg[:], in_=k_tab[:])

            kp_view = k_tab[:].rearrange("(a b) c -> a (b c)", b=2)
            vnp_view = vn_tab[:].rearrange("(a b) c -> a (b c)", b=2)

            # ---- pass 1: dot, exp, den scatter ----
            ex_tiles = []
            for t in list(range(NT if not done else 0)) * rep:
                qi = wk.tile([128, T // 16], I16, tag="qi")
                ki = wk.tile([128, T // 16], I16, tag="ki")
                di = wk.tile([128, T // 16], I16, tag="di")
                pr = wk.tile([128, T // 128], FP, tag="pr")
                nc.sync.dma_start(out=qi[:], in_=qg_idx[t])
                nc.sync.dma_start(out=ki[:], in_=kp_idx[t])
                nc.sync.dma_start(out=di[:], in_=den_idx[t])
                nc.sync.dma_start(out=pr[:], in_=par_in[t])

                qe = wk.tile([128, T // 128, F], FP, tag="qe")
                kp = wk.tile([128, T // 128, 2 * F], FP, tag="pairg")
                for g in range(T // GC):
                    s16, s128 = GC // 16, GC // 128
                    nc.gpsimd.dma_gather(
                        qe[:, g * s128:(g + 1) * s128],
                        q_tab[:], qi[:, g * s16:(g + 1) * s16], GC, GC, F)
                    nc.gpsimd.dma_gather(
                        kp[:, g * s128:(g + 1) * s128],
                        kp_view, ki[:, g * s16:(g + 1) * s16], GC, GC, 2 * F,
                        elem_step=2 * F)

                tmp = wk.tile([128, T // 128, H, D], FP, tag="tmp")
                dotL = wk.tile([128, T // 128, H], FP, tag="dotL")
                dotH = wk.tile([128, T // 128, H], FP, tag="dotH")
                qe4 = qe[:].rearrange("p a (h d) -> p a h d", d=D)
                nc.vector.tensor_tensor(
                    out=tmp[:], in0=qe4,
                    in1=kp[:, :, 0:F].rearrange("p a (h d) -> p a h d", d=D),
                    op=mybir.AluOpType.mult)
                nc.vector.tensor_reduce(
                    out=dotL[:], in_=tmp[:], axis=mybir.AxisListType.X,
                    op=mybir.AluOpType.add)
                nc.vector.tensor_tensor(
                    out=tmp[:], in0=qe4,
                    in1=kp[:, :, F:2 * F].rearrange("p a (h d) -> p a h d", d=D),
                    op=mybir.AluOpType.mult)
                nc.vector.tensor_reduce(
                    out=dotH[:], in_=tmp[:], axis=mybir.AxisListType.X,
                    op=mybir.AluOpType.add)
                # dot = dotL + par * (dotH - dotL)
                nc.vector.tensor_sub(out=dotH[:], in0=dotH[:], in1=dotL[:])
                nc.vector.tensor_tensor(
                    out=dotH[:], in0=dotH[:],
                    in1=pr[:].unsqueeze(-1).to_broadcast([128, T // 128, H]),
                    op=mybir.AluOpType.mult)
                nc.vector.tensor_add(out=dotL[:], in0=dotL[:], in1=dotH[:])
                ex_t = exp_pool.tile([128, T // 128, H], FP, tag=f"ex{t}")
                nc.scalar.activation(ex_t[:], dotL[:],
                                     mybir.ActivationFunctionType.Exp,
                                     scale=SCALE)
                if len(ex_tiles) < NT:
                    ex_tiles.append(ex_t)

                half = 0 if t < nt_lo else 1
                tl = t if half == 0 else t - nt_lo
                nc.gpsimd.dma_scatter_add(
                    den_st[half][tl % S_DEN][:, 0:4], ex_t[:], di[:], T, T, 4,
                    elem_step=64)

            # ---- den reduce + allreduce ----
            for j0 in range(0, 2 * SPLIT if not done else 0, 512):
                half = 0 if j0 < SPLIT else 1
                r0 = j0 - half * SPLIT
                a = wk.tile([128, 4, 64], FP, tag="dena")
                b = wk.tile([128, 4, 64], FP, tag="denb")
                nc.sync.dma_start(
                    out=a[:], in_=den_st[half][0][r0:r0 + 512, :].rearrange(
                        "(a b) c -> b a c", b=128))
                nc.sync.dma_start(
                    out=b[:], in_=den_st[half][1][r0:r0 + 512, :].rearrange(
                        "(a b) c -> b a c", b=128))
                nc.vector.tensor_add(out=a[:, :, 0:4], in0=a[:, :, 0:4],
                                     in1=b[:, :, 0:4])
                nc.sync.dma_start(
                    out=den4[j0:j0 + 512, :].rearrange("(a b) c -> b a c", b=128),
                    in_=a[:, :, 0:4])

            if stage in ("pass1", "tp1") and not done:
                if stage == "pass1":
                    nc.sync.dma_start(out=dbg[:], in_=den4[:])
                done = True

            if not done:
                if stage == "sim":
                    nc.sync.dma_start(out=den4_ar[:], in_=den4[:])
                else:
                    nc.gpsimd.collective_compute(
                        "AllReduce", mybir.AluOpType.add,
                        replica_groups=[list(range(CORES))],
                        ins=[den4[:]], outs=[den4_ar[:]],
                    )
            if stage in ("ar", "tar") and not done:
                if stage == "ar":
                    nc.sync.dma_start(out=dbg[:], in_=den4_ar[:])
                done = True

            # ---- vn = v / den ----
            for j0 in range(0, NODEP if not done else 0, 512):
                vt = wk.tile([128, 4, F], FP, tag="vt")
                dt_ = wk.tile([128, 4, 4], FP, tag="dent")
                nc.sync.dma_start(
                    out=vt[:], in_=v_tab[j0:j0 + 512, :].rearrange(
                        "(a b) c -> b a c", b=128))
                if j0 < 2 * SPLIT:
                    nc.sync.dma_start(
                        out=dt_[:], in_=den4_ar[j0:j0 + 512, :].rearrange(
                            "(a b) c -> b a c", b=128))
                else:
                    nc.vector.memset(dt_[:], 1.0)
                nc.vector.tensor_scalar_max(out=dt_[:], in0=dt_[:], scalar1=1e-30)
                nc.vector.reciprocal(out=dt_[:], in_=dt_[:])
                nc.vector.tensor_tensor(
                    out=vt[:].rearrange("p a (h d) -> p a h d", d=D),
                    in0=vt[:].rearrange("p a (h d) -> p a h d", d=D),
                    in1=dt_[:].unsqueeze(-1).to_broadcast([128, 4, 4, D]),
                    op=mybir.AluOpType.mult)
                nc.sync.dma_start(
                    out=vn_tab[j0:j0 + 512, :].rearrange("(a b) c -> b a c", b=128),
                    in_=vt[:])

            if stage in ("vn", "tvn") and not done:
                if stage == "vn":
                    nc.sync.dma_start(out=dbg[:], in_=vn_tab[:])
                done = True

            # ---- pass 2: messages + res scatter ----
            for t in list(range(NT if not done else 0)) * rep:
                ki = wk.tile([128, T // 16], I16, tag="ki")
                qi = wk.tile([128, T // 16], I16, tag="qi")
                pr = wk.tile([128, T // 128], FP, tag="pr")
                nc.sync.dma_start(out=ki[:], in_=kp_idx[t])
                nc.sync.dma_start(out=qi[:], in_=qg_idx[t])
                nc.sync.dma_start(out=pr[:], in_=par_in[t])
                vnp = wk.tile([128, T // 128, 2 * F], FP, tag="pairg")
                for g in range(T // GC):
                    s16, s128 = GC // 16, GC // 128
                    nc.gpsimd.dma_gather(
                        vnp[:, g * s128:(g + 1) * s128],
                        vnp_view, ki[:, g * s16:(g + 1) * s16], GC, GC, 2 * F,
                        elem_step=2 * F)
                ex_t = ex_tiles[t]
                exH = wk.tile([128, T // 128, H], FP, tag="exH")
                exL = wk.tile([128, T // 128, H], FP, tag="exL")
                nc.vector.tensor_tensor(
                    out=exH[:], in0=ex_t[:],
                    in1=pr[:].unsqueeze(-1).to_broadcast([128, T // 128, H]),
                    op=mybir.AluOpType.mult)
                nc.vector.tensor_sub(out=exL[:], in0=ex_t[:], in1=exH[:])
                msg = wk.tile([128, T // 128, H, D], FP, tag="tmp")
                msg2 = wk.tile([128, T // 128, H, D], FP, tag="tmp2")
                nc.vector.tensor_tensor(
                    out=msg[:],
                    in0=vnp[:, :, 0:F].rearrange("p a (h d) -> p a h d", d=D),
                    in1=exL[:].unsqueeze(-1).to_broadcast([128, T // 128, H, D]),
                    op=mybir.AluOpType.mult)
                nc.vector.tensor_tensor(
                    out=msg2[:],
                    in0=vnp[:, :, F:2 * F].rearrange("p a (h d) -> p a h d", d=D),
                    in1=exH[:].unsqueeze(-1).to_broadcast([128, T // 128, H, D]),
                    op=mybir.AluOpType.mult)
                nc.vector.tensor_add(out=msg[:], in0=msg[:], in1=msg2[:])
                msgf = msg[:].rearrange("p a h d -> p a (h d)")
                for g in range(T // GC):
                    s16, s128 = GC // 16, GC // 128
                    nc.gpsimd.dma_scatter_add(
                        res_st[t % S_RES][:],
                        msgf[:, g * s128:(g + 1) * s128],
                        qi[:, g * s16:(g + 1) * s16], GC, GC, F)

            # ---- output: res = sum stripes; out = res @ Wo (+bo) ----
            for j in range(QROWS // 128 if not done else 1):
                racc = wk.tile([128, F], FP, tag="racc")
                nc.sync.dma_start(out=racc[:],
                                  in_=res_st[0][j * 128:(j + 1) * 128, :])
                for s in range(1, S_RES):
                    rb = wk.tile([128, F], FP, tag="rb")
                    nc.sync.dma_start(out=rb[:],
                                      in_=res_st[s][j * 128:(j + 1) * 128, :])
                    nc.vector.tensor_add(out=racc[:], in0=racc[:], in1=rb[:])
                pst = pp2.tile([128, 128], FP, tag="pst")
                nc.tensor.transpose(out=pst[:], in_=racc[:], identity=ident[:])
                rT = wk.tile([128, F], FP, tag="rT")
                nc.vector.tensor_copy(out=rT[:], in_=pst[:])
                po = pp2.tile([128, 128], FP, tag="po")
                nc.tensor.matmul(out=po[:], lhsT=rT[:], rhs=wo_t[:],
                                 start=True, stop=not with_bias)
                if with_bias:
                    nc.tensor.matmul(out=po[:], lhsT=ones1[:], rhs=bias_t[3:4],
                                     start=False, stop=True)
                ot = wk.tile([128, F], FP, tag="ot")
                nc.vector.tensor_copy(out=ot[:], in_=po[:])
                nc.sync.dma_start(out=out_c[j * 128:(j + 1) * 128, :], in_=ot[:])

    nc.compile()
    return nc


# ---------------- PJRT SPMD runner (embedded) ----------------
class SpmdRunner:
    def __init__(self, nc, n_cores=8):
        import jax
        from jax.sharding import Mesh, NamedSharding, PartitionSpec
        from jax.experimental.shard_map import shard_map
        from concourse.bass2jax import (
            _bass_exec_p, install_neuronx_cc_hook, partition_id_tensor)

        self.jax = jax
        install_neuronx_cc_hook()
        self.nc = nc
        self.n_cores = n_cores
        pname = nc.partition_id_tensor.name if nc.partition_id_tensor else None
        in_names, out_names, out_avals, zero_shapes = [], [], [], []
        for alloc in nc.m.functions[0].allocations:
            if not isinstance(alloc, mybir.MemoryLocationSet):
                continue
            name = alloc.memorylocations[0].name
            if alloc.kind == "ExternalInput":
                if name != pname:
                    in_names.append(name)
            elif alloc.kind == "ExternalOutput":
                shape = tuple(alloc.tensor_shape)
                dtype = mybir.dt.np(alloc.dtype)
                out_names.append(name)
                out_avals.append(jax.core.ShapedArray(shape, dtype))
                zero_shapes.append((shape, dtype))
        self.in_names = in_names
        self.out_names = out_names
        self.out_avals = out_avals
        self.zero_shapes = zero_shapes
        n_params = len(in_names)
        n_outs = len(out_avals)
        all_in = in_names + out_names + ([pname] if pname else [])
        donate = tuple(range(n_params, n_params + n_outs))

        def _body(*args):
            operands = list(args)
            if pname is not None:
                operands.append(partition_id_tensor())
            return tuple(_bass_exec_p.bind(
                *operands, out_avals=tuple(out_avals), in_names=tuple(all_in),
                out_names=tuple(out_names), lowering_input_output_aliases=(),
                sim_require_finite=False, sim_require_nnan=False, nc=nc))

        devices = jax.devices()[:n_cores]
        self.mesh = Mesh(np.asarray(devices), ("core",))
        in_specs = (PartitionSpec("core"),) * (n_params + n_outs)
        out_specs = (PartitionSpec("core"),) * n_outs
        self.fn = jax.jit(
            shard_map(_body, mesh=self.mesh, in_specs=in_specs,
                      out_specs=out_specs, check_rep=False),
            donate_argnums=donate, keep_unused=True)
        self.sharding = NamedSharding(self.mesh, PartitionSpec("core"))

    def put_inputs(self, in_maps):
        concat = [
            np.concatenate([np.asarray(m[name]) for m in in_maps], axis=0)
            for name in self.in_names
        ]
        return [self.jax.device_put(a, self.sharding) for a in concat]

    def _zeros(self):
        return [
            self.jax.device_put(
                np.zeros((self.n_cores * s[0], *s[1:]), d), self.sharding)
            for s, d in self.zero_shapes
        ]

    def run_raw(self, dev_args):
        outs = self.fn(*dev_args, *self._zeros())
        self.jax.block_until_ready(outs)
        return outs

    def to_results(self, outs):
        return [
            {
                name: np.asarray(outs[i]).reshape(
                    self.n_cores, *self.out_avals[i].shape)[c]
                for i, name in enumerate(self.out_names)
            }
            for c in range(self.n_cores)
        ]

    def run_results(self, dev_args):
        return self.to_results(self.run_raw(dev_args))


# ---------------- top level ----------------
_CACHE = {}


def _get_runner(nt_lo, nt_hi, with_bias):
    key = (nt_lo, nt_hi, with_bias)
    if key not in _CACHE:
        nc = build_nc(nt_lo, nt_hi, with_bias)
        _CACHE[key] = SpmdRunner(nc, n_cores=CORES)
    return _CACHE[key]


def make_in_maps(query, memory, edge_rows, edge_cols, Wq, bq, Wk, bk, Wv, bv,
                 Wo, bo):
    query = np.asarray(query, np.float32)
    memory = np.asarray(memory, np.float32)
    per_core_idx, nt_lo, nt_hi = host_prep(np.asarray(edge_rows),
                                           np.asarray(edge_cols))
    memT = np.zeros((F, NODEP), np.float32)
    memT[:, :M] = np.asarray(memory, np.float32).T
    biases = np.stack([np.asarray(b, np.float32) for b in (bq, bk, bv, bo)])
    with_bias = bool(np.abs(biases).max() > 0)
    common = dict(
        memoryT=memT,
        Wq=np.asarray(Wq, np.float32), Wk=np.asarray(Wk, np.float32),
        Wv=np.asarray(Wv, np.float32), Wo=np.asarray(Wo, np.float32),
        biases=biases,
    )
    in_maps = []
    for c in range(CORES):
        qT = np.zeros((F, QROWS), np.float32)
        qT[:, :RPC] = query[c * RPC:(c + 1) * RPC].T
        m = dict(common)
        m["queryT"] = qT
        m.update(per_core_idx[c])
        in_maps.append(m)
    return in_maps, nt_lo, nt_hi, with_bias


def kernel(query, memory, edge_rows, edge_cols, Wq, bq, Wk, bk, Wv, bv, Wo, bo):
    in_maps, nt_lo, nt_hi, with_bias = make_in_maps(
        query, memory, edge_rows, edge_cols, Wq, bq, Wk, bk, Wv, bv, Wo, bo)
    run = _get_runner(nt_lo, nt_hi, with_bias)
    dev = run.put_inputs(in_maps)
    res = run.run_results(dev)
    return np.concatenate([res[c]["out"][:RPC] for c in range(CORES)], axis=0)



# revision 31
# speedup vs baseline: 25.8432x; 25.8432x over previous
"""Trainium2 Bass kernel for nn_DotAttentionLayer (edge-parallel sparse GNN attention).

Self-contained: takes FULL inputs (query, memory, edge_rows, edge_cols, Wq, bq,
Wk, bk, Wv, bv, Wo, bo), returns the FULL [50000, 128] output.

Strategy (8 cores, SPMD):
  - Edges sharded by row-owner core (row // 6250); node tables replicated.
  - Host colors each core's edges into 2048-edge tiles such that within a tile
    all rows are unique AND all cols are unique (scatter-add duplicates within
    one DMA instruction race on HW; across instructions Tile's WAW serializes).
  - Tiles are split by col half (< / >= 25088) so k/vn gathers use half-local
    int16 indices against fp16 lo/hi table slices (256B rows).
  - Pass 1: gather q,k rows per edge (fp16), dot per head, exp -> den
    scatter-add (fp32, 4 col-stripes inside one 64-wide table).  AllReduce den
    (800KB).  vn phase recomputes v from memoryT and folds 1/den, writing a
    fp16 vn table.  Pass 2: gather vn, msg = ex * vn (fp16), scatter-add into
    2 fp16 res stripes.  out = res @ Wo.
"""

import sys

sys.path.insert(0, "/opt/trn_rl_repo")
import numpy as np

import concourse.bacc as bacc
import concourse.mybir as mybir
import concourse.tile as tile
from concourse import library_config
from concourse.bass import AP  # noqa
from concourse.masks import make_identity

# ---------------- problem constants ----------------
N = 50000
M = 50000
E = 1000000
F = 128
H = 4
D = 32
SCALE = float(1.0 / np.sqrt(D))
LRELU_ALPHA = 0.2

CORES = 8
RPC = N // CORES            # 6250 rows per core
QROWS = 6272                # 49*128, row-gather table size (trash row = RPC)
Q_TRASH = RPC
NODEP = 50176               # 392*128 padded node count
SPLIT = 25088               # col half split (196*128); NODEP == 2*SPLIT
DENROWS = 25216             # 197*128; trash row below
DEN_TRASH_LO = SPLIT        # local trash row for lo half
DEN_TRASH_HI = 25000        # local trash row for hi half (real hi locals < 24912)
T = 2048                    # edges per tile
GC = 1024                   # max indices per dma_gather call (HW ring limit)
S_DEN = 2                   # den scatter stripes (WAW pipelining)
S_RES = 2                   # res scatter stripes
FP = mybir.dt.float32
F16 = mybir.dt.float16
I16 = mybir.dt.int16


# ---------------- host-side edge coloring ----------------
def _color_half(rows, cols, cap=T):
    """Bipartite edge coloring, least-loaded free color greedy: tiles are
    matchings (unique rows AND cols) capped at `cap` edges.  Returns list of
    np.ndarray of edge indices (into rows/cols)."""
    n = len(rows)
    rmask = np.zeros(RPC + 1, np.int64)      # per-row used-color bitset
    cmask = np.zeros(SPLIT, np.int64)        # per-col used-color bitset
    count = np.zeros(64, np.int32)
    color = np.empty(n, np.int32)
    deg = (np.bincount(rows, minlength=RPC + 1)[rows]
           + np.bincount(cols, minlength=SPLIT)[cols])
    order = np.argsort(-deg)
    hi = 1
    for e in order:
        r, c = rows[e], cols[e]
        used = int(rmask[r] | cmask[c])
        best = -1
        bestc = 1 << 30
        for k in range(hi):
            if not ((used >> k) & 1) and count[k] < cap and count[k] < bestc:
                best, bestc = k, count[k]
        if best < 0:
            best = hi
            hi += 1
        bit = 1 << best
        rmask[r] |= bit
        cmask[c] |= bit
        count[best] += 1
        color[e] = best
    return [np.nonzero(color == k)[0] for k in range(hi)]


def _wrap16(idx_2d):
    """[nt, T] -> [nt, 128, T//16] int16 wrap-16 layout replicated to 128."""
    nt = idx_2d.shape[0]
    w = idx_2d.reshape(nt, T // 16, 16).transpose(0, 2, 1)  # [nt,16,T//16]
    return np.tile(w, (1, 8, 1)).astype(np.int16)


def host_prep(edge_rows, edge_cols):
    """Shard + color edges. Returns per-core index arrays + (NT_LO, NT_HI)."""
    edge_rows = np.asarray(edge_rows)
    edge_cols = np.asarray(edge_cols)
    owner = edge_rows // RPC
    per_core = []
    for c in range(CORES):
        m = owner == c
        r = (edge_rows[m] - c * RPC).astype(np.int64)
        col = edge_cols[m].astype(np.int64)
        lo = col < SPLIT
        tiles_lo = _color_half(r[lo], col[lo])
        tiles_hi = _color_half(r[~lo], col[~lo] - SPLIT)
        per_core.append((r, col, lo, tiles_lo, tiles_hi))
    nt_lo = max(len(p[3]) for p in per_core)
    nt_hi = max(len(p[4]) for p in per_core)

    inputs = []
    for c in range(CORES):
        r, col, lo, tiles_lo, tiles_hi = per_core[c]
        idx_lo = np.nonzero(lo)[0]
        idx_hi = np.nonzero(~lo)[0]
        nt = nt_lo + nt_hi
        rl = np.full((nt, T), Q_TRASH, np.int64)
        kl = np.zeros((nt, T), np.int64)  # k/vn gather idx, half-local
        dl = np.zeros((nt, T), np.int64)
        for t in range(nt_lo):
            dl[t] = DEN_TRASH_LO
        for t in range(nt_hi):
            dl[nt_lo + t] = DEN_TRASH_HI
        for t, tl in enumerate(tiles_lo):
            e = idx_lo[tl]
            rl[t, : len(e)] = r[e]
            kl[t, : len(e)] = col[e]
            dl[t, : len(e)] = col[e]
        for t, tl in enumerate(tiles_hi):
            e = idx_hi[tl]
            rl[nt_lo + t, : len(e)] = r[e]
            kl[nt_lo + t, : len(e)] = col[e] - SPLIT
            dl[nt_lo + t, : len(e)] = col[e] - SPLIT
        # one [NT, 128, 3, T//16] tensor: (q, k, den) idx interleaved so a
        # tile's load is one contiguous 768B-per-partition DMA
        idx3 = np.stack([_wrap16(rl), _wrap16(kl), _wrap16(dl)], axis=2)
        inputs.append(dict(idx3=idx3))
    return inputs, nt_lo, nt_hi


# ---------------- device program ----------------
def build_nc(nt_lo, nt_hi, with_bias, stage="full"):
    NT = nt_lo + nt_hi
    nc = bacc.Bacc("TRN2", debug=False)
    dbg = None
    if stage in ("dense", "vn"):
        dbg = nc.dram_tensor("dbg", [NODEP, F], FP, kind="ExternalOutput")
    elif stage in ("pass1", "ar"):
        dbg = nc.dram_tensor("dbg", [2 * SPLIT, 4], FP, kind="ExternalOutput")

    # I/O (fp16 feature inputs prepared on host)
    queryT = nc.dram_tensor("queryT", [F, QROWS], F16, kind="ExternalInput")
    memoryT = nc.dram_tensor("memoryT", [F, NODEP], F16, kind="ExternalInput")
    Wq = nc.dram_tensor("Wq", [F, F], F16, kind="ExternalInput")
    Wk = nc.dram_tensor("Wk", [F, F], F16, kind="ExternalInput")
    Wv = nc.dram_tensor("Wv", [F, F], F16, kind="ExternalInput")
    Wo = nc.dram_tensor("Wo", [F, F], FP, kind="ExternalInput")
    biases = nc.dram_tensor("biases", [4, F], F16, kind="ExternalInput")
    bo_in = nc.dram_tensor("bo", [1, F], FP, kind="ExternalInput")
    idx3 = nc.dram_tensor("idx3", [NT, 128, 3, T // 16], I16, kind="ExternalInput")
    out_c = nc.dram_tensor("out", [QROWS, F], FP, kind="ExternalOutput")

    # internal DRAM (k/vn split by col half so cross-phase deps are precise)
    q_tab = nc.dram_tensor("q_tab", [QROWS, F], F16)
    k_half = [nc.dram_tensor(f"k_{h}", [SPLIT, F], F16) for h in range(2)]
    vn_half = [nc.dram_tensor(f"vn_{h}", [SPLIT, F], F16) for h in range(2)]
    den_st = [
        [nc.dram_tensor(f"den_{h}_{s}", [DENROWS, 64], FP) for s in range(S_DEN)]
        for h in range(2)
    ]
    den4 = nc.dram_tensor("den4", [2 * SPLIT, 4], FP)
    den4_ar = nc.dram_tensor("den4_ar", [2 * SPLIT, 4], FP)
    res_st = [nc.dram_tensor(f"res_{s}", [QROWS, F], F16) for s in range(S_RES)]

    with tile.TileContext(nc) as tc:
        with tc.tile_pool(name="static", bufs=1) as st, \
             tc.tile_pool(name="work", bufs=3) as wk, \
             tc.tile_pool(name="exp", bufs=1) as exp_pool, \
             tc.tile_pool(name="psum", bufs=3, space="PSUM") as pp, \
             tc.tile_pool(name="psum2", bufs=2, space="PSUM") as pp2:
            nc.gpsimd.load_library(library_config.mlp)

            # ---- static tiles ----
            zt = st.tile([128, 2048], FP, tag="zero")
            nc.vector.memset(zt[:], 0.0)
            ident = st.tile([128, 128], FP, tag="ident")
            make_identity(nc, ident[:])
            wq_t = st.tile([F, F], F16, tag="wq")
            wk_t = st.tile([F, F], F16, tag="wk")
            wv_t = st.tile([F, F], F16, tag="wv")
            wo_t = st.tile([F, F], FP, tag="wo")
            nc.sync.dma_start(out=wq_t[:], in_=Wq[:])
            nc.sync.dma_start(out=wk_t[:], in_=Wk[:])
            nc.sync.dma_start(out=wv_t[:], in_=Wv[:])
            nc.sync.dma_start(out=wo_t[:], in_=Wo[:])
            if with_bias:
                ones1 = st.tile([1, 128], F16, tag="ones1")
                nc.vector.memset(ones1[:], 1.0)
                ones1f = st.tile([1, 128], FP, tag="ones1f")
                nc.vector.memset(ones1f[:], 1.0)
                bias_t = st.tile([4, F], F16, tag="bias")
                nc.sync.dma_start(out=bias_t[:], in_=biases[:])
                bo_t = st.tile([1, F], FP, tag="bo")
                nc.sync.dma_start(out=bo_t[:], in_=bo_in[:])

            # ---- zero the accumulator stripes ----
            def zero_dram(tensor, rows, width, dt=FP):
                per = rows * width // 128  # elems per partition
                ap = tensor[:].rearrange("(a b) c -> a (b c)", a=128)
                zv = zt[:] if dt == FP else zt[:].bitcast(F16)
                off = 0
                zn = 2048 if dt == FP else 4096
                while off < per:
                    n = min(zn, per - off)
                    nc.sync.dma_start(out=ap[:, off:off + n], in_=zv[:, 0:n])
                    off += n

            # den stripes: only columns 0:4 are ever written/read - zero just
            # those via a strided DMA (16B runs, descriptor-count bound)
            z4 = zt[:, 0:DENROWS // 128 * 4].rearrange("p (a c) -> p a c", c=4)
            for h in range(2):
                for s in range(S_DEN):
                    nc.sync.dma_start(
                        out=den_st[h][s][:, 0:4].rearrange(
                            "(b a) c -> b a c", b=128),
                        in_=z4)
            for s in range(S_RES):
                zero_dram(res_st[s], QROWS, F, F16)

            # ---- dense phase: fp16 tables via fp16 matmul + ACT lrelu ----
            # lhsT is restrided so matmul `a` computes rows {p*4+a}: partition
            # p then holds 4 consecutive table rows -> contiguous 1KB DMA runs
            def dense(srcT, c0, ncols, w_t, b_row, dst):
                for j0 in range(c0, c0 + ncols, 512):
                    nn_ = min(512, c0 + ncols - j0)
                    na = nn_ // 128
                    xt = wk.tile([128, 512], F16, tag="xT")
                    nc.sync.dma_start(out=xt[:, 0:nn_], in_=srcT[:, j0:j0 + nn_])
                    xtv = xt[:, 0:nn_].rearrange("f (b a) -> f a b", a=na)
                    sb = wk.tile([128, 4, 128], F16, tag="dsb")
                    ps4 = pp.tile([128, 4, 128], FP, tag="dps")
                    for a in range(na):
                        nc.tensor.matmul(
                            out=ps4[:, a],
                            lhsT=xtv[:, a],
                            rhs=w_t[:],
                            start=True, stop=not with_bias,
                        )
                        if with_bias:
                            nc.tensor.matmul(
                                out=ps4[:, a], lhsT=ones1[:], rhs=b_row,
                                start=False, stop=True,
                            )
                    nc.scalar.activation(
                        sb[:, 0:na], ps4[:, 0:na],
                        mybir.ActivationFunctionType.Prelu,
                        alpha=LRELU_ALPHA)
                    nc.sync.dma_start(
                        out=dst[j0 - c0:j0 - c0 + nn_, :].rearrange(
                            "(b a) c -> b (a c)", b=128, a=na),
                        in_=sb[:, 0:na].rearrange("p a c -> p (a c)"),
                    )

            bq_r = bias_t[0:1] if with_bias else None
            bk_r = bias_t[1:2] if with_bias else None
            dense(queryT, 0, QROWS, wq_t, bq_r, q_tab)
            dense(memoryT, 0, SPLIT, wk_t, bk_r, k_half[0])
            dense(memoryT, SPLIT, SPLIT, wk_t, bk_r, k_half[1])

            done = stage in ("dense", "noop")
            if stage == "dense":
                for h in range(2):
                    nc.sync.dma_start(
                        out=dbg[h * SPLIT:(h + 1) * SPLIT].bitcast(F16)[:, 0:F],
                        in_=k_half[h][:])

            # ---- pass 1: dot, exp, den scatter ----
            ex_tiles = []
            for t in range(NT if not done else 0):
                ix = wk.tile([128, 3, T // 16], I16, tag="ix")
                nc.scalar.dma_start(out=ix[:], in_=idx3[t])

                half = 0 if t < nt_lo else 1
                qe = wk.tile([128, T // 128, F], F16, tag="qe")
                ke = wk.tile([128, T // 128, F], F16, tag="ke")
                for g in range(T // GC):
                    s16, s128 = GC // 16, GC // 128
                    nc.gpsimd.dma_gather(
                        qe[:, g * s128:(g + 1) * s128], q_tab[:],
                        ix[:, 0, g * s16:(g + 1) * s16], GC, GC, F)
                    nc.gpsimd.dma_gather(
                        ke[:, g * s128:(g + 1) * s128], k_half[half][:],
                        ix[:, 1, g * s16:(g + 1) * s16], GC, GC, F)

                tmp = wk.tile([128, T // 128, H, D], F16, tag="tmp")
                dotL = wk.tile([128, T // 128, H], FP, tag="dotL")
                nc.vector.tensor_tensor(
                    out=tmp[:],
                    in0=qe[:].rearrange("p a (h d) -> p a h d", d=D),
                    in1=ke[:].rearrange("p a (h d) -> p a h d", d=D),
                    op=mybir.AluOpType.mult)
                nc.vector.tensor_reduce(
                    out=dotL[:], in_=tmp[:], axis=mybir.AxisListType.X,
                    op=mybir.AluOpType.add)
                ex_t = exp_pool.tile([128, T // 128, H], FP, tag=f"ex{t}")
                nc.scalar.activation(ex_t[:], dotL[:],
                                     mybir.ActivationFunctionType.Exp,
                                     scale=SCALE)
                ex_tiles.append(ex_t)

                tl = t if half == 0 else t - nt_lo
                nc.gpsimd.dma_scatter_add(
                    den_st[half][tl % S_DEN][:, 0:4], ex_t[:], ix[:, 2], T, T, 4,
                    elem_step=64)

            # ---- den reduce (one big strided read per stripe) + allreduce ----
            for half in range(2 if not done else 0):
                a = wk.tile([128, SPLIT // 128, 4], FP, tag="dena")
                b = wk.tile([128, SPLIT // 128, 4], FP, tag="denb")
                # b-major: partition b holds den rows b*197.. (row-block view);
                # matches the contiguous den4 write below
                nc.scalar.dma_start(
                    out=a[:], in_=den_st[half][0][0:SPLIT, 0:4].rearrange(
                        "(b a) c -> b a c", b=128))
                nc.scalar.dma_start(
                    out=b[:], in_=den_st[half][1][0:SPLIT, 0:4].rearrange(
                        "(b a) c -> b a c", b=128))
                nc.vector.tensor_add(out=a[:], in0=a[:], in1=b[:])
                nc.sync.dma_start(
                    out=den4[half * SPLIT:(half + 1) * SPLIT, :].rearrange(
                        "(b a) c -> b (a c)", b=128),
                    in_=a[:].rearrange("p a c -> p (a c)"))

            if stage == "pass1" and not done:
                nc.sync.dma_start(out=dbg[:], in_=den4[:])
                done = True

            if not done:
                if stage == "sim":
                    nc.sync.dma_start(out=den4_ar[:], in_=den4[:])
                else:
                    nc.gpsimd.collective_compute(
                        "AllReduce", mybir.AluOpType.add,
                        replica_groups=[list(range(CORES))],
                        ins=[den4[:]], outs=[den4_ar[:]],
                    )
            if stage == "ar" and not done:
                nc.sync.dma_start(out=dbg[:], in_=den4_ar[:])
                done = True

            # ---- per col half: vn = lrelu(memT @ Wv + bv) / den (fp16),
            # then pass 2 for that half's tiles (overlaps other half's vn) ----
            for half in range(2 if not done else 0):
                for j0 in range(half * SPLIT, (half + 1) * SPLIT, 512):
                    xt = wk.tile([128, 512], F16, tag="xT")
                    nc.sync.dma_start(out=xt[:], in_=memoryT[:, j0:j0 + 512])
                    xtv = xt[:].rearrange("f (b a) -> f a b", a=4)
                    vt = wk.tile([128, 4, F], FP, tag="vt")
                    ps4 = pp.tile([128, 4, 128], FP, tag="dps")
                    for a in range(4):
                        nc.tensor.matmul(
                            out=ps4[:, a], lhsT=xtv[:, a],
                            rhs=wv_t[:], start=True, stop=not with_bias)
                        if with_bias:
                            nc.tensor.matmul(
                                out=ps4[:, a], lhsT=ones1[:], rhs=bias_t[2:3],
                                start=False, stop=True)
                    nc.scalar.activation(
                        vt[:], ps4[:],
                        mybir.ActivationFunctionType.Prelu,
                        alpha=LRELU_ALPHA)
                    # den for rows {j0 + p*4 + a} at [p, a, :]
                    rden = wk.tile([128, 4, 4], FP, tag="rden")
                    nc.scalar.dma_start(
                        out=rden[:],
                        in_=den4_ar[j0:j0 + 512, :].rearrange(
                            "(b a) c -> b a c", b=128))
                    nc.vector.tensor_scalar_max(out=rden[:], in0=rden[:],
                                                scalar1=1e-30)
                    nc.vector.reciprocal(out=rden[:], in_=rden[:])
                    vn16 = wk.tile([128, 4, F], F16, tag="vn16")
                    nc.vector.tensor_tensor(
                        out=vn16[:].rearrange("p a (h d) -> p a h d", d=D),
                        in0=vt[:].rearrange("p a (h d) -> p a h d", d=D),
                        in1=rden[:].unsqueeze(-1).to_broadcast(
                            [128, 4, 4, D]),
                        op=mybir.AluOpType.mult)
                    nc.sync.dma_start(
                        out=vn_half[half][j0 - half * SPLIT:
                                          j0 - half * SPLIT + 512, :].rearrange(
                            "(b a) c -> b (a c)", b=128),
                        in_=vn16[:].rearrange("p a c -> p (a c)"))

                if stage == "vn":
                    continue

                # ---- pass 2 for this half: messages + res scatter ----
                t0_, t1_ = (0, nt_lo) if half == 0 else (nt_lo, NT)
                for t in range(t0_, t1_):
                    ix2 = wk.tile([128, 2, T // 16], I16, tag="ix2")
                    nc.scalar.dma_start(out=ix2[:], in_=idx3[t][:, 0:2])
                    ve = wk.tile([128, T // 128, F], F16, tag="ve")
                    for g in range(T // GC):
                        s16, s128 = GC // 16, GC // 128
                        nc.gpsimd.dma_gather(
                            ve[:, g * s128:(g + 1) * s128], vn_half[half][:],
                            ix2[:, 1, g * s16:(g + 1) * s16], GC, GC, F)
                    ex_t = ex_tiles[t]
                    msg = wk.tile([128, T // 128, H, D], F16, tag="msg")
                    nc.vector.tensor_tensor(
                        out=msg[:],
                        in0=ve[:].rearrange("p a (h d) -> p a h d", d=D),
                        in1=ex_t[:].unsqueeze(-1).to_broadcast(
                            [128, T // 128, H, D]),
                        op=mybir.AluOpType.mult)
                    msgf = msg[:].rearrange("p a h d -> p a (h d)")
                    for g in range(T // GC):
                        s16, s128 = GC // 16, GC // 128
                        nc.gpsimd.dma_scatter_add(
                            res_st[t % S_RES][:],
                            msgf[:, g * s128:(g + 1) * s128],
                            ix2[:, 0, g * s16:(g + 1) * s16], GC, GC, F)

            if stage == "vn" and not done:
                for h in range(2):
                    nc.sync.dma_start(
                        out=dbg[h * SPLIT:(h + 1) * SPLIT].bitcast(F16)[:, 0:F],
                        in_=vn_half[h][:])
                done = True

            # ---- output: res = sum stripes; out = res @ Wo (+bo) ----
            for j in range(QROWS // 128 if not done else 1):
                r0 = wk.tile([128, F], F16, tag="racc0")
                r1 = wk.tile([128, F], F16, tag="racc1")
                nc.sync.dma_start(out=r0[:],
                                  in_=res_st[0][j * 128:(j + 1) * 128, :])
                nc.sync.dma_start(out=r1[:],
                                  in_=res_st[1][j * 128:(j + 1) * 128, :])
                racc = wk.tile([128, F], FP, tag="racc")
                nc.vector.tensor_add(out=racc[:], in0=r0[:], in1=r1[:])
                pst = pp2.tile([128, 128], FP, tag="pst")
                nc.tensor.transpose(out=pst[:], in_=racc[:], identity=ident[:])
                rT = wk.tile([128, F], FP, tag="rT")
                nc.vector.tensor_copy(out=rT[:], in_=pst[:])
                po = pp2.tile([128, 128], FP, tag="po")
                nc.tensor.matmul(out=po[:], lhsT=rT[:], rhs=wo_t[:],
                                 start=True, stop=not with_bias)
                if with_bias:
                    nc.tensor.matmul(out=po[:], lhsT=ones1f[:], rhs=bo_t[:],
                                     start=False, stop=True)
                ot = wk.tile([128, F], FP, tag="ot")
                nc.vector.tensor_copy(out=ot[:], in_=po[:])
                nc.sync.dma_start(out=out_c[j * 128:(j + 1) * 128, :], in_=ot[:])

    nc.compile()
    return nc


# ---------------- PJRT SPMD runner (embedded) ----------------
class SpmdRunner:
    def __init__(self, nc, n_cores=8):
        import jax
        from jax.sharding import Mesh, NamedSharding, PartitionSpec
        from jax.experimental.shard_map import shard_map
        from concourse.bass2jax import (
            _bass_exec_p, install_neuronx_cc_hook, partition_id_tensor)

        self.jax = jax
        install_neuronx_cc_hook()
        self.nc = nc
        self.n_cores = n_cores
        pname = nc.partition_id_tensor.name if nc.partition_id_tensor else None
        in_names, out_names, out_avals, zero_shapes = [], [], [], []
        for alloc in nc.m.functions[0].allocations:
            if not isinstance(alloc, mybir.MemoryLocationSet):
                continue
            name = alloc.memorylocations[0].name
            if alloc.kind == "ExternalInput":
                if name != pname:
                    in_names.append(name)
            elif alloc.kind == "ExternalOutput":
                shape = tuple(alloc.tensor_shape)
                dtype = mybir.dt.np(alloc.dtype)
                out_names.append(name)
                out_avals.append(jax.core.ShapedArray(shape, dtype))
                zero_shapes.append((shape, dtype))
        self.in_names = in_names
        self.out_names = out_names
        self.out_avals = out_avals
        self.zero_shapes = zero_shapes
        n_params = len(in_names)
        n_outs = len(out_avals)
        all_in = in_names + out_names + ([pname] if pname else [])
        donate = tuple(range(n_params, n_params + n_outs))

        def _body(*args):
            operands = list(args)
            if pname is not None:
                operands.append(partition_id_tensor())
            return tuple(_bass_exec_p.bind(
                *operands, out_avals=tuple(out_avals), in_names=tuple(all_in),
                out_names=tuple(out_names), lowering_input_output_aliases=(),
                sim_require_finite=False, sim_require_nnan=False, nc=nc))

        devices = jax.devices()[:n_cores]
        self.mesh = Mesh(np.asarray(devices), ("core",))
        in_specs = (PartitionSpec("core"),) * (n_params + n_outs)
        out_specs = (PartitionSpec("core"),) * n_outs
        self.fn = jax.jit(
            shard_map(_body, mesh=self.mesh, in_specs=in_specs,
                      out_specs=out_specs, check_rep=False),
            donate_argnums=donate, keep_unused=True)
        self.sharding = NamedSharding(self.mesh, PartitionSpec("core"))

    def put_inputs(self, in_maps):
        concat = [
            np.concatenate([np.asarray(m[name]) for m in in_maps], axis=0)
            for name in self.in_names
        ]
        return [self.jax.device_put(a, self.sharding) for a in concat]

    def _zeros(self):
        return [
            self.jax.device_put(
                np.zeros((self.n_cores * s[0], *s[1:]), d), self.sharding)
            for s, d in self.zero_shapes
        ]

    def run_raw(self, dev_args):
        outs = self.fn(*dev_args, *self._zeros())
        self.jax.block_until_ready(outs)
        return outs

    def to_results(self, outs):
        return [
            {
                name: np.asarray(outs[i]).reshape(
                    self.n_cores, *self.out_avals[i].shape)[c]
                for i, name in enumerate(self.out_names)
            }
            for c in range(self.n_cores)
        ]

    def run_results(self, dev_args):
        return self.to_results(self.run_raw(dev_args))


# ---------------- top level ----------------
_CACHE = {}


def _get_runner(nt_lo, nt_hi, with_bias):
    key = (nt_lo, nt_hi, with_bias)
    if key not in _CACHE:
        nc = build_nc(nt_lo, nt_hi, with_bias)
        _CACHE[key] = SpmdRunner(nc, n_cores=CORES)
    return _CACHE[key]


def make_in_maps(query, memory, edge_rows, edge_cols, Wq, bq, Wk, bk, Wv, bv,
                 Wo, bo):
    query = np.asarray(query, np.float32)
    memory = np.asarray(memory, np.float32)
    per_core_idx, nt_lo, nt_hi = host_prep(np.asarray(edge_rows),
                                           np.asarray(edge_cols))
    memT = np.zeros((F, NODEP), np.float16)
    memT[:, :M] = np.asarray(memory, np.float32).T.astype(np.float16)
    biases = np.stack([np.asarray(b, np.float32) for b in (bq, bk, bv, bo)])
    with_bias = bool(np.abs(biases).max() > 0)
    common = dict(
        memoryT=memT,
        Wq=np.asarray(Wq, np.float16), Wk=np.asarray(Wk, np.float16),
        Wv=np.asarray(Wv, np.float16), Wo=np.asarray(Wo, np.float32),
        biases=biases.astype(np.float16),
        bo=np.asarray(bo, np.float32).reshape(1, F),
    )
    in_maps = []
    for c in range(CORES):
        qT = np.zeros((F, QROWS), np.float16)
        qT[:, :RPC] = query[c * RPC:(c + 1) * RPC].T.astype(np.float16)
        m = dict(common)
        m["queryT"] = qT
        m.update(per_core_idx[c])
        in_maps.append(m)
    return in_maps, nt_lo, nt_hi, with_bias


def kernel(query, memory, edge_rows, edge_cols, Wq, bq, Wk, bk, Wv, bv, Wo, bo):
    in_maps, nt_lo, nt_hi, with_bias = make_in_maps(
        query, memory, edge_rows, edge_cols, Wq, bq, Wk, bk, Wv, bv, Wo, bo)
    run = _get_runner(nt_lo, nt_hi, with_bias)
    dev = run.put_inputs(in_maps)
    res = run.run_results(dev)
    return np.concatenate([res[c]["out"][:RPC] for c in range(CORES)], axis=0)
